# revision 1
# baseline (speedup 1.0000x reference)
"""Trainium2 Bass kernel: single-head causal attention with RoPE,
sharded across 8 NeuronCores (2 cores per batch element).

kernel(**inputs) takes the FULL inputs (x [4,4096,1024], wq/wk/wv
[1024,1024], all fp32) and returns the FULL output [4,4096,1024] fp32.
Work split: core c handles batch c//2 and eight 256-row query
superblocks chosen so every core needs key-extent exactly 512*(i+1) in
local slot i (identical program on all cores); causal masks arrive as
per-core data and the mask-rounding waste is split evenly between the
two cores of each batch.
"""

import sys
for _p in ("/root/.axon_site", "/root/.axon_site/_ro/trn_rl_repo",
           "/root/.axon_site/_ro/pypackages"):
    if _p not in sys.path:
        sys.path.append(_p)

"""Single-head causal attention with RoPE, sharded over 2*B NeuronCores.

Sharding: core c handles batch b=c//2, and the q-row 256-superblocks with
parity h=c%2 (global superblock g = 2i+h for local i).  Every core runs an
IDENTICAL program shape: local superblock i has key-extent 512*(i+1); the
causality differences between parities are absorbed by per-core mask data.

K/V projections (USE_CC=True): each core projects only its own key half
(even core: keys [0,S/2), odd core: [S/2,S)), then a pair AllGather moves
both halves to both cores through DRAM bounce buffers; DMA-in places
group-rank-0's half at the low key range on every core, so KT_sb/V_sb end
up in global key order with fully uniform addressing.

Layouts on device (partition dim first):
  KT_sb [128, 8, SEQ]    : K^T, partition = feature e%128, chunk e//128 (RoPE'd, perm)
  V_sb  [128, SEQ//128, 1025] : V, partition = key k%128; col 1024 = 1.0
  QT    [128, 8, 256]    : per-super Q^T (RoPE'd, perm)
Scores computed transposed: S^T[k, q] = sum_e KT[e,k] * QT[e,q] so that
P^T = exp(S^T) feeds the PV matmul as the stationary operand directly.
The softmax denominator rides along as V's 1025th ones-column (third PV
matmul N=341 whose last output column is sum_k P^T).  Softmax runs without
max-subtraction (scores are O(6); exp is safe in fp32).
"""

import numpy as np

import concourse.bass as bass
import concourse.bacc as bacc
import concourse.mybir as mybir
from concourse import tile

P = 128
D = 1024
DC = 8          # feature chunks of 128
W = 256         # projection s-block width
SUP_Q = 256     # q rows per superblock
ROPE_BASE = 10000.0
SCALE = 1.0 / 32.0   # 1/sqrt(D)
DV1 = 1025      # V free width incl. ones column
PV_SPLIT = [(0, 342), (342, 684), (684, 1025)]

BF = mybir.dt.bfloat16
F32 = mybir.dt.float32
AF = mybir.ActivationFunctionType


def build_kernel(nc, SEQ, use_cc=True):
    NSUP = SEQ // (2 * SUP_Q)
    NKC = SEQ // P
    QROWS = SEQ // 2
    KHALF = SEQ // 2 if use_cc else SEQ

    xk = nc.dram_tensor("xk", [D, KHALF], BF, kind="ExternalInput")
    xq = nc.dram_tensor("xq", [D, QROWS], BF, kind="ExternalInput")
    wkT = nc.dram_tensor("wkT", [D, D], BF, kind="ExternalInput")
    wqT = nc.dram_tensor("wqT", [D, D], BF, kind="ExternalInput")
    wvT = nc.dram_tensor("wvT", [D, D], BF, kind="ExternalInput")
    cosk = nc.dram_tensor("cosk", [D // 2, KHALF], BF, kind="ExternalInput")
    sink = nc.dram_tensor("sink", [D // 2, KHALF], BF, kind="ExternalInput")
    cosq = nc.dram_tensor("cosq", [D // 2, QROWS], BF, kind="ExternalInput")
    sinq = nc.dram_tensor("sinq", [D // 2, QROWS], BF, kind="ExternalInput")
    maskI = nc.dram_tensor("mask", [NSUP, 4 * P, SUP_Q], BF, kind="ExternalInput")
    out = nc.dram_tensor("out", [QROWS, D], F32, kind="ExternalOutput")

    xk_r = xk.rearrange("(c p) s -> p c s", p=P)
    xq_r = xq.rearrange("(c p) s -> p c s", p=P)
    wk_r = wkT.rearrange("(c p) e -> p c e", p=P)
    wq_r = wqT.rearrange("(c p) e -> p c e", p=P)
    wv_r = wvT.rearrange("(c p) e -> p c e", p=P)
    cosk_r = cosk.rearrange("(c p) s -> p c s", p=P)
    sink_r = sink.rearrange("(c p) s -> p c s", p=P)
    cosq_r = cosq.rearrange("(c p) s -> p c s", p=P)
    sinq_r = sinq.rearrange("(c p) s -> p c s", p=P)
    mask_r = maskI.rearrange("n (c p) q -> p n c q", p=P)

    cc = None
    if use_cc:
        kt_out = nc.dram_tensor("kt_out", [DC, P, KHALF], BF)
        kt_g = nc.dram_tensor("kt_g", [2, DC, P, KHALF], BF)
        v_out = nc.dram_tensor("v_out", [KHALF // P, P, D], BF)
        v_g = nc.dram_tensor("v_g", [2, KHALF // P, P, D], BF)
        ngroups = max(1, nc.num_devices // 2)
        groups = [[2 * g, 2 * g + 1] for g in range(ngroups)]
        cc = (kt_out, kt_g, v_out, v_g, groups)

    with tile.TileContext(nc) as tc:
        _emit(tc, nc, SEQ, KHALF, NSUP, NKC,
              xk_r, xq_r, wk_r, wq_r, wv_r,
              cosk_r, sink_r, cosq_r, sinq_r, mask_r, out, cc)
    return nc


def _rope_evict(nc, pool, pse, pso, cos_ap, sin_ap, out_e_ap, out_o_ap, width, tag):
    """out_e = e*cos - o*sin ; out_o = o*cos + e*sin (psum fp32 -> sbuf bf16)."""
    ce = pool.tile([P, width], BF, tag=f"{tag}ce")
    nc.scalar.copy(ce[:], pse[:])
    co = pool.tile([P, width], BF, tag=f"{tag}co")
    nc.scalar.copy(co[:], pso[:])
    me = pool.tile([P, width], BF, tag=f"{tag}me")
    nc.vector.tensor_mul(me[:], ce[:], cos_ap)
    mo = pool.tile([P, width], BF, tag=f"{tag}mo")
    nc.vector.tensor_mul(mo[:], co[:], sin_ap)
    nc.vector.tensor_sub(out_e_ap, me[:], mo[:])
    me2 = pool.tile([P, width], BF, tag=f"{tag}me2")
    nc.vector.tensor_mul(me2[:], ce[:], sin_ap)
    mo2 = pool.tile([P, width], BF, tag=f"{tag}mo2")
    nc.vector.tensor_mul(mo2[:], co[:], cos_ap)
    nc.vector.tensor_add(out_o_ap, me2[:], mo2[:])


def _load_w_chunked(nc, pool, w_r, tag):
    tiles = []
    for dc in range(DC):
        t = pool.tile([P, D], BF, tag=f"{tag}{dc}", name=f"{tag}_{dc}")
        nc.sync.dma_start(t[:], w_r[:, dc, :])
        tiles.append(t)
    return tiles


def _emit(tc, nc, SEQ, KHALF, NSUP, NKC,
          xk_r, xq_r, wk_r, wq_r, wv_r,
          cosk_r, sink_r, cosq_r, sinq_r, mask_r, out, cc):
    NSB = KHALF // W
    with (
        tc.tile_pool(name="kt", bufs=1) as ktp,
        tc.tile_pool(name="v", bufs=1) as vp,
        tc.tile_pool(name="wq", bufs=1) as wqp,
    ):
        KT_sb = ktp.tile([P, DC, SEQ], BF, tag="KT")
        V_sb = vp.tile([P, NKC, DV1], BF, tag="V")
        nc.vector.memset(V_sb[:, :, 1024:1025], 1.0)

        # ---------- Phase A: K+V projections over this core's key range ----------
        with (
            tc.tile_pool(name="wkv", bufs=1) as wkvp,
            tc.tile_pool(name="xs", bufs=3) as xsp,
            tc.tile_pool(name="cs", bufs=2) as csp,
            tc.tile_pool(name="ev", bufs=2) as evp,
            tc.tile_pool(name="kvs", bufs=2) as kvsp,
            tc.tile_pool(name="pa", bufs=6, space="PSUM") as pap,
        ):
            # first-block streams, then weights (wq prefetched for phase C)
            xt0 = xsp.tile([P, DC, W], BF, tag="xk", name="xt_0")
            nc.sync.dma_start(xt0[:], xk_r[:, :, 0:W])
            ck0 = csp.tile([P, 4, W], BF, tag="ck", name="ck_0")
            nc.sync.dma_start(ck0[:], cosk_r[:, :, 0:W])
            sk0 = csp.tile([P, 4, W], BF, tag="sk", name="sk_0")
            nc.sync.dma_start(sk0[:], sink_r[:, :, 0:W])
            wk_t = _load_w_chunked(nc, wkvp, wk_r, "wk")
            wv_t = _load_w_chunked(nc, wkvp, wv_r, "wv")
            wq_t = _load_w_chunked(nc, wqp, wq_r, "wq")

            if cc is None:
                for sb in range(NSB):
                    s0 = sb * W
                    if sb == 0:
                        xt, ck, sk = xt0, ck0, sk0
                    else:
                        xt = xsp.tile([P, DC, W], BF, tag="xk", name=f"xt_{sb}")
                        nc.sync.dma_start(xt[:], xk_r[:, :, s0:s0 + W])
                        ck = csp.tile([P, 4, W], BF, tag="ck", name=f"ck_{sb}")
                        nc.sync.dma_start(ck[:], cosk_r[:, :, s0:s0 + W])
                        sk = csp.tile([P, 4, W], BF, tag="sk", name=f"sk_{sb}")
                        nc.sync.dma_start(sk[:], sink_r[:, :, s0:s0 + W])
                    _emit_kblock(nc, evp, pap, wk_t, xt, ck, sk, KT_sb, None, s0)
                    _emit_vblock(nc, pap, wv_t, xt, V_sb, None, sb)
            else:
                kt_out, kt_g, v_out, v_g, groups = cc
                kt_out_r = kt_out.rearrange("c p s -> p c s")
                v_out_r = v_out.rearrange("k p d -> p k d")
                # pass 1: K projection over own half
                for sb in range(NSB):
                    s0 = sb * W
                    if sb == 0:
                        xt, ck, sk = xt0, ck0, sk0
                    else:
                        xt = xsp.tile([P, DC, W], BF, tag="xk", name=f"xt_{sb}")
                        nc.sync.dma_start(xt[:], xk_r[:, :, s0:s0 + W])
                        ck = csp.tile([P, 4, W], BF, tag="ck", name=f"ck_{sb}")
                        nc.sync.dma_start(ck[:], cosk_r[:, :, s0:s0 + W])
                        sk = csp.tile([P, 4, W], BF, tag="sk", name=f"sk_{sb}")
                        nc.sync.dma_start(sk[:], sink_r[:, :, s0:s0 + W])
                    _emit_kblock(nc, evp, pap, wk_t, xt, ck, sk, KT_sb,
                                 (kvsp, kt_out_r), s0)
                nc.gpsimd.collective_compute(
                    "AllGather", mybir.AluOpType.bypass,
                    replica_groups=groups, ins=[kt_out[:]], outs=[kt_g[:]])
                kt_g_r = kt_g.rearrange("g c p s -> p g c s")
                engs = [nc.sync, nc.scalar]
                n = 0
                for g in range(2):
                    for dc in range(DC):
                        for piece in range(2):
                            pw = KHALF // 2
                            dst0 = g * KHALF + piece * pw
                            engs[n % 2].dma_start(
                                KT_sb[:, dc, dst0:dst0 + pw],
                                kt_g_r[:, g, dc, piece * pw:(piece + 1) * pw])
                            n += 1
                # pass 2: V projection over own half
                for sb in range(NSB):
                    s0 = sb * W
                    xt = xsp.tile([P, DC, W], BF, tag="xk", name=f"xtv_{sb}")
                    nc.sync.dma_start(xt[:], xk_r[:, :, s0:s0 + W])
                    _emit_vblock(nc, pap, wv_t, xt, V_sb, (kvsp, v_out_r), sb)
                nc.gpsimd.collective_compute(
                    "AllGather", mybir.AluOpType.bypass,
                    replica_groups=groups, ins=[v_out[:]], outs=[v_g[:]])
                v_g_r = v_g.rearrange("g k p d -> p g k d")
                n = 0
                for g in range(2):
                    for kl in range(KHALF // P):
                        engs[n % 2].dma_start(
                            V_sb[:, g * (KHALF // P) + kl, 0:D],
                            v_g_r[:, g, kl, :])
                        n += 1

        # ---------- Phase C: per-superblock Q projection + attention ----------
        with (
            tc.tile_pool(name="xq", bufs=2) as xqp,
            tc.tile_pool(name="cq", bufs=2) as cqp,
            tc.tile_pool(name="qt", bufs=2) as qtp,
            tc.tile_pool(name="evq", bufs=3) as evqp,
            tc.tile_pool(name="pt", bufs=6) as ptp,
            tc.tile_pool(name="mk", bufs=2) as mkp,
            tc.tile_pool(name="ot", bufs=3) as otp,
            tc.tile_pool(name="rd", bufs=2) as rdp,
            tc.tile_pool(name="pq", bufs=2, space="PSUM") as pqp,
            tc.tile_pool(name="po", bufs=1, space="PSUM") as pop,
        ):
            for i in range(NSUP):
                q0 = i * SUP_Q
                xqt = xqp.tile([P, DC, SUP_Q], BF, tag="xq")
                nc.sync.dma_start(xqt[:], xq_r[:, :, q0:q0 + SUP_Q])
                cq = cqp.tile([P, 4, SUP_Q], BF, tag="cq")
                nc.sync.dma_start(cq[:], cosq_r[:, :, q0:q0 + SUP_Q])
                sq = cqp.tile([P, 4, SUP_Q], BF, tag="sq")
                nc.sync.dma_start(sq[:], sinq_r[:, :, q0:q0 + SUP_Q])
                QT = qtp.tile([P, DC, SUP_Q], BF, tag="QT")
                for j in range(4):
                    pse = pqp.tile([P, SUP_Q], F32, tag="pq")
                    for dc in range(DC):
                        nc.tensor.matmul(pse[:], wq_t[dc][:, j * P:(j + 1) * P],
                                         xqt[:, dc, :],
                                         start=(dc == 0), stop=(dc == DC - 1))
                    pso = pqp.tile([P, SUP_Q], F32, tag="pq")
                    for dc in range(DC):
                        nc.tensor.matmul(pso[:], wq_t[dc][:, (j + 4) * P:(j + 5) * P],
                                         xqt[:, dc, :],
                                         start=(dc == 0), stop=(dc == DC - 1))
                    _rope_evict(nc, evqp, pse, pso,
                                cq[:, j, :], sq[:, j, :],
                                QT[:, j, :], QT[:, j + 4, :],
                                SUP_Q, "q")

                nkc = 4 * (i + 1)
                mk = mkp.tile([P, 4, SUP_Q], BF, tag="mk")
                nc.sync.dma_start(mk[:], mask_r[:, i, :, :])
                o_ps = [pop.tile([P, sl[1] - sl[0]], F32, tag=f"po{n}",
                                 name=f"o_ps{i}_{n}")
                        for n, sl in enumerate(PV_SPLIT + PV_SPLIT)]

                pending = None
                for kc in range(nkc):
                    ps_s = pqp.tile([P, SUP_Q], F32, tag="pq")
                    for dc in range(DC):
                        nc.tensor.matmul(ps_s[:], KT_sb[:, dc, kc * P:(kc + 1) * P],
                                         QT[:, dc, :],
                                         start=(dc == 0), stop=(dc == DC - 1))
                    pt = ptp.tile([P, SUP_Q], BF, tag="pt")
                    nc.scalar.activation(pt[:], ps_s[:], AF.Exp, scale=SCALE)
                    if kc >= nkc - 4:
                        nc.vector.tensor_mul(pt[:], pt[:], mk[:, kc - (nkc - 4), :])
                    if pending is not None:
                        _emit_pv(nc, pending, V_sb, o_ps, nkc)
                    pending = (pt, kc)
                _emit_pv(nc, pending, V_sb, o_ps, nkc)

                rd = rdp.tile([P, 2], F32, tag="rd")
                nc.vector.reciprocal(rd[:, 0:1], o_ps[2][:, 340:341])
                nc.vector.reciprocal(rd[:, 1:2], o_ps[5][:, 340:341])
                for qs in range(2):
                    ot = otp.tile([P, D], F32, tag="ot")
                    for n, (a, b) in enumerate(PV_SPLIT):
                        bb = min(b, D)
                        nc.vector.tensor_scalar_mul(
                            ot[:, a:bb], o_ps[qs * 3 + n][:, 0:bb - a],
                            rd[:, qs:qs + 1])
                    r0 = q0 + qs * P
                    nc.sync.dma_start(out[r0:r0 + P, :], ot[:])



def _emit_kblock(nc, evp, pap, wk_t, xt, ck, sk, KT_sb, stage, s0):
    for j in range(4):
        pse = pap.tile([P, W], F32, tag="ps")
        for dc in range(DC):
            nc.tensor.matmul(pse[:], wk_t[dc][:, j * P:(j + 1) * P],
                             xt[:, dc, :],
                             start=(dc == 0), stop=(dc == DC - 1))
        pso = pap.tile([P, W], F32, tag="ps")
        for dc in range(DC):
            nc.tensor.matmul(pso[:], wk_t[dc][:, (j + 4) * P:(j + 5) * P],
                             xt[:, dc, :],
                             start=(dc == 0), stop=(dc == DC - 1))
        if stage is None:
            _rope_evict(nc, evp, pse, pso, ck[:, j, :], sk[:, j, :],
                        KT_sb[:, j, s0:s0 + W], KT_sb[:, j + 4, s0:s0 + W],
                        W, "k")
        else:
            kvsp, kt_out_r = stage
            se = kvsp.tile([P, W], BF, tag="se")
            so = kvsp.tile([P, W], BF, tag="so")
            _rope_evict(nc, evp, pse, pso, ck[:, j, :], sk[:, j, :],
                        se[:], so[:], W, "k")
            nc.sync.dma_start(kt_out_r[:, j, s0:s0 + W], se[:])
            nc.sync.dma_start(kt_out_r[:, j + 4, s0:s0 + W], so[:])


def _emit_vblock(nc, pap, wv_t, xt, V_sb, stage, sb):
    for sc in range(W // P):
        kc = sb * (W // P) + sc
        for half in range(2):
            psv = pap.tile([P, 512], F32, tag="ps")
            for dc in range(DC):
                nc.tensor.matmul(psv[:],
                                 xt[:, dc, sc * P:(sc + 1) * P],
                                 wv_t[dc][:, half * 512:(half + 1) * 512],
                                 start=(dc == 0), stop=(dc == DC - 1))
            if stage is None:
                nc.scalar.copy(V_sb[:, kc, half * 512:(half + 1) * 512], psv[:])
            else:
                kvsp, v_out_r = stage
                vs = kvsp.tile([P, 512], BF, tag="vs")
                nc.scalar.copy(vs[:], psv[:])
                nc.sync.dma_start(v_out_r[:, kc, half * 512:(half + 1) * 512],
                                  vs[:])


def _emit_pv(nc, pending, V_sb, o_ps, nkc):
    pt, kc = pending
    first = (kc == 0)
    last = (kc == nkc - 1)
    for qs in range(2):
        lhs = pt[:, qs * P:(qs + 1) * P]
        for n, (a, b) in enumerate(PV_SPLIT):
            nc.tensor.matmul(o_ps[qs * 3 + n][:], lhs, V_sb[:, kc, a:b],
                             start=first, stop=last)


# ---------------------------------------------------------------------------
# Host-side data preparation
# ---------------------------------------------------------------------------

def rope_tables(seq_len):
    pos = np.arange(seq_len, dtype=np.float64)
    inv = ROPE_BASE ** (-np.arange(0, D, 2, dtype=np.float64) / D)
    fr = inv[:, None] * pos[None, :]
    return np.cos(fr).astype(np.float32), np.sin(fr).astype(np.float32)


def perm_indices():
    return np.concatenate([np.arange(0, D, 2), np.arange(1, D, 2)])


def g_map(NSUP, i, h):
    """Global superblock held in local slot i on parity-h cores.  The first
    half of slots takes parity h, the second half parity 1-h: both cores
    then need extent exactly 512*(i+1) per slot and the mask-rounding waste
    splits evenly instead of landing all on one parity."""
    return 2 * i + h if i < NSUP // 2 else 2 * i + (1 - h)


def q_indices(SEQ, h):
    NSUP = SEQ // (2 * SUP_Q)
    return np.concatenate(
        [np.arange(SUP_Q * g_map(NSUP, i, h), SUP_Q * g_map(NSUP, i, h) + SUP_Q)
         for i in range(NSUP)])


def make_masks(SEQ, h):
    NSUP = SEQ // (2 * SUP_Q)
    m = np.zeros((NSUP, 4 * P, SUP_Q), dtype=np.float32)
    for i in range(NSUP):
        E = 512 * (i + 1)
        keys = E - 512 + np.arange(512)
        qrows = SUP_Q * g_map(NSUP, i, h) + np.arange(SUP_Q)
        m[i] = (keys[:, None] <= qrows[None, :]).astype(np.float32)
    return m


def prep_all(x, wq, wk, wv, use_cc=True):
    import ml_dtypes
    bf = ml_dtypes.bfloat16
    B, SEQ, _ = x.shape
    pi = perm_indices()
    wqT_p = np.ascontiguousarray(wq[pi, :].T).astype(bf)
    wkT_p = np.ascontiguousarray(wk[pi, :].T).astype(bf)
    wvT = np.ascontiguousarray(wv.T).astype(bf)
    cos_t, sin_t = rope_tables(SEQ)
    cos_b, sin_b = cos_t.astype(bf), sin_t.astype(bf)
    in_maps = []
    for c in range(2 * B):
        b, h = c // 2, c % 2
        xT = np.ascontiguousarray(x[b].T).astype(bf)
        qi = q_indices(SEQ, h)
        if use_cc:
            k0, k1 = h * SEQ // 2, (h + 1) * SEQ // 2
        else:
            k0, k1 = 0, SEQ
        in_maps.append({
            "xk": np.ascontiguousarray(xT[:, k0:k1]),
            "xq": np.ascontiguousarray(xT[:, qi]),
            "wkT": wkT_p, "wqT": wqT_p, "wvT": wvT,
            "cosk": np.ascontiguousarray(cos_b[:, k0:k1]),
            "sink": np.ascontiguousarray(sin_b[:, k0:k1]),
            "cosq": np.ascontiguousarray(cos_b[:, qi]),
            "sinq": np.ascontiguousarray(sin_b[:, qi]),
            "mask": make_masks(SEQ, h).astype(bf),
        })
    return in_maps


def assemble_output(results, B, SEQ):
    NSUP = SEQ // (2 * SUP_Q)
    out = np.empty((B, SEQ, D), dtype=np.float32)
    for c in range(2 * B):
        b, h = c // 2, c % 2
        o = results[c]["out"]
        for i in range(NSUP):
            g = g_map(NSUP, i, h)
            out[b, SUP_Q * g:SUP_Q * (g + 1), :] = o[SUP_Q * i:SUP_Q * (i + 1), :]
    return out


# ---------------------------------------------------------------------------
# Numpy reference (port of reference.py)
# ---------------------------------------------------------------------------

def np_reference(x, wq, wk, wv):
    B, S, d = x.shape
    q = np.einsum('bsd,ed->bse', x, wq)
    k = np.einsum('bsd,ed->bse', x, wk)
    v = np.einsum('bsd,ed->bse', x, wv)

    pos = np.arange(S, dtype=np.float64)[:, None]
    inv = 1.0 / ROPE_BASE ** (np.arange(0, d, 2, dtype=np.float64) / d)
    fr = pos * inv[None, :]
    cos = np.repeat(np.cos(fr), 2, axis=-1).astype(np.float32)
    sin = np.repeat(np.sin(fr), 2, axis=-1).astype(np.float32)

    def rot(t):
        e, o = t[..., 0::2], t[..., 1::2]
        return np.stack((-o, e), axis=-1).reshape(t.shape)

    q = q * cos + rot(q) * sin
    k = k * cos + rot(k) * sin

    scores = np.einsum('bqd,bkd->bqk', q, k) / np.sqrt(d)
    mask = np.triu(np.ones((S, S), dtype=bool), k=1)
    scores = np.where(mask, -np.inf, scores)
    scores -= scores.max(axis=-1, keepdims=True)
    w = np.exp(scores)
    w /= w.sum(axis=-1, keepdims=True)
    return np.einsum('bqk,bkd->bqd', w, v).astype(np.float32)


# ---------------------------------------------------------------------------
# Entry point
# ---------------------------------------------------------------------------

USE_CC = False
_COMPILED = {}


def _get_compiled(SEQ, n_cores, use_cc):
    key = (SEQ, n_cores, use_cc)
    if key not in _COMPILED:
        nc = bacc.Bacc("TRN2", target_bir_lowering=False, debug=False,
                       num_devices=n_cores)
        build_kernel(nc, SEQ, use_cc=use_cc)
        nc.compile()
        _COMPILED[key] = nc
    return _COMPILED[key]


def kernel(x, wq, wk, wv):
    from concourse.bass_utils import run_bass_kernel_spmd
    x = np.asarray(x, dtype=np.float32)
    wq = np.asarray(wq, dtype=np.float32)
    wk = np.asarray(wk, dtype=np.float32)
    wv = np.asarray(wv, dtype=np.float32)
    B, SEQ, d = x.shape
    assert d == D
    n_cores = 2 * B
    nc = _get_compiled(SEQ, n_cores, USE_CC)
    in_maps = prep_all(x, wq, wk, wv, use_cc=USE_CC)
    res = run_bass_kernel_spmd(nc, in_maps, list(range(n_cores)))
    return assemble_output(res.results, B, SEQ)

